# revision 5
# baseline (speedup 1.0000x reference)
"""BP-MLL loss kernel for Trainium2, data-parallel over 8 NeuronCores.

Math: the reference loss is
    L = mean_b  (1/(n_pos_b * n_neg_b)) * sum_{k in Y_b, l in Ybar_b} exp(c_bl - c_bk)
The pairwise sum is separable:
    sum_{k,l} yf_k * ybar_l * exp(c_l) * exp(-c_k)
      = (sum_l ybar_l * exp(c_l)) * (sum_k yf_k * exp(-c_k))
so each batch row only needs two masked exp-sums (O(L) instead of O(L^2)).

Sharding: B=32 rows split 4-per-core across 8 cores. On each core, its
[4, 2048] shard is viewed as [128 partitions, 64 free] (row b on
partitions 32b..32b+31). Fused ACT/DVE reduce ops produce four
per-partition partial sums (sum e^c, sum yf*e^c, sum yf*e^-c, sum yf);
the tiny [128, 4] stats tensor goes back to the host, which finishes
the per-row reduction, the division by n_pos*n_neg, and the mean.

Written in raw Bass (explicit semaphores): the TileContext tail drain
emits a multi-wait Drain instruction that this container's walrus
rejects ("Too many sync wait commands").
"""

import sys
from contextlib import ExitStack

import numpy as np

for _p in ("/opt/trn_rl_repo",):
    if _p not in sys.path:
        sys.path.append(_p)

B, L = 32, 2048
N_CORES = 8
B_SHARD = B // N_CORES  # 4 batch rows per core
Q = 128 // B_SHARD  # 32 partitions per batch row
J = L // Q  # 64 free elements per partition

_CACHE = {}


def _build_bass():
    import concourse.bass as bass
    from concourse import mybir

    F32 = mybir.dt.float32
    I32 = mybir.dt.int32
    Exp = mybir.ActivationFunctionType.Exp
    mult = mybir.AluOpType.mult
    AX = mybir.AxisListType.X

    nc = bass.Bass()
    c_in = nc.declare_dram_parameter("c", [B_SHARD, L], F32, isOutput=False)
    y_in = nc.declare_dram_parameter("y", [B_SHARD, L], I32, isOutput=False)
    out = nc.declare_dram_parameter("stats", [128, 4], F32, isOutput=True)

    # Pure reshape of the contiguous [4, 2048] shard to [128, 64].
    c_view = c_in[:].rearrange("b (q j) -> (b q) j", j=J)
    y_view = y_in[:].rearrange("b (q j) -> (b q) j", j=J)

    with ExitStack() as es:
        c_sb = es.enter_context(nc.sbuf_tensor([128, J], F32))
        y_sb = es.enter_context(nc.sbuf_tensor([128, J], I32))
        yf = es.enter_context(nc.sbuf_tensor([128, J], F32))
        e_pos = es.enter_context(nc.sbuf_tensor([128, J], F32))
        e_neg = es.enter_context(nc.sbuf_tensor([128, J], F32))
        prod1 = es.enter_context(nc.sbuf_tensor([128, J], F32))
        prod2 = es.enter_context(nc.sbuf_tensor([128, J], F32))
        stats = es.enter_context(nc.sbuf_tensor([128, 4], F32))

        c_sem = es.enter_context(nc.semaphore("c_sem"))
        y_sem = es.enter_context(nc.semaphore("y_sem"))
        act_sem = es.enter_context(nc.semaphore("act_sem"))
        dve_sem = es.enter_context(nc.semaphore("dve_sem"))
        out_sem = es.enter_context(nc.semaphore("out_sem"))

        block = es.enter_context(nc.Block())

        @block.sync
        def _(sync):
            sync.dma_start(out=c_sb[:], in_=c_view).then_inc(c_sem, 16)
            sync.dma_start(out=y_sb[:], in_=y_view).then_inc(y_sem, 16)
            # stats cols: 0 from ACT (first activation), 1-3 from DVE
            sync.wait_ge(act_sem, 1)
            sync.wait_ge(dve_sem, 4)
            sync.dma_start(out=out[:], in_=stats[:]).then_inc(out_sem, 16)
            sync.wait_ge(out_sem, 16)

        @block.scalar
        def _(scalar):
            scalar.wait_ge(c_sem, 16)
            # e_pos = exp(c); accum -> stats[:,0] = sum_j exp(c)
            scalar.activation(
                out=e_pos[:], in_=c_sb[:], func=Exp, accum_out=stats[:, 0:1]
            ).then_inc(act_sem, 1)
            # e_neg = exp(-c)
            scalar.activation(
                out=e_neg[:], in_=c_sb[:], func=Exp, scale=-1.0
            ).then_inc(act_sem, 2)

        @block.vector
        def _(vector):
            vector.wait_ge(y_sem, 16)
            vector.tensor_copy(out=yf[:], in_=y_sb[:])  # int32 -> f32 0/1 mask
            # stats[:,3] = sum_j yf
            vector.reduce_sum(out=stats[:, 3:4], in_=yf[:], axis=AX).then_inc(
                dve_sem, 2
            )
            vector.wait_ge(act_sem, 1)
            # stats[:,1] = sum_j yf * exp(c)
            vector.tensor_tensor(out=prod1[:], in0=yf[:], in1=e_pos[:], op=mult)
            vector.reduce_sum(out=stats[:, 1:2], in_=prod1[:], axis=AX).then_inc(
                dve_sem, 1
            )
            vector.wait_ge(act_sem, 3)
            # stats[:,2] = sum_j yf * exp(-c)
            vector.tensor_tensor(out=prod2[:], in0=yf[:], in1=e_neg[:], op=mult)
            vector.reduce_sum(out=stats[:, 2:3], in_=prod2[:], axis=AX).then_inc(
                dve_sem, 1
            )

    return nc


def _get_nc():
    if "nc" not in _CACHE:
        _CACHE["nc"] = _build_bass()
    return _CACHE["nc"]


def _run_device(c, y, trace=False):
    from concourse.bass_utils import run_bass_kernel_spmd

    in_maps = [
        {
            "c": np.ascontiguousarray(c[i * B_SHARD : (i + 1) * B_SHARD], np.float32),
            "y": np.ascontiguousarray(y[i * B_SHARD : (i + 1) * B_SHARD], np.int32),
        }
        for i in range(N_CORES)
    ]
    return run_bass_kernel_spmd(
        _get_nc(), in_maps, core_ids=list(range(N_CORES)), trace=trace
    )


def _combine(results):
    """results: per-core dicts with 'stats' [128, 4] f32."""
    total = 0.0
    for r in results:
        s = r["stats"].astype(np.float64).reshape(B_SHARD, Q, 4).sum(axis=1)
        s_epos, s_pos_epos, s_pos_eneg, n_pos = s.T
        s1 = s_epos - s_pos_epos  # sum((1-yf) * exp(c))
        n_neg = L - n_pos
        total += float(np.sum(s1 * s_pos_eneg / (n_pos * n_neg)))
    return np.float32(total / B)


def kernel(c, y):
    res = _run_device(np.asarray(c), np.asarray(y))
    return np.asarray(_combine(res.results), dtype=np.float32)


# revision 10
# speedup vs baseline: 1.2924x; 1.2924x over previous
"""BP-MLL loss kernel for Trainium2, data-parallel over 8 NeuronCores.

Math: the reference loss is
    L = mean_b  (1/(n_pos_b * n_neg_b)) * sum_{k in Y_b, l in Ybar_b} exp(c_bl - c_bk)
The pairwise sum is separable:
    sum_{k,l} yf_k * ybar_l * exp(c_l) * exp(-c_k)
      = (sum_l ybar_l * exp(c_l)) * (sum_k yf_k * exp(-c_k))  =  S1_b * S2_b
so each batch row only needs two masked exp-sums (O(L) instead of O(L^2)).

The masks fold into the exponent (exp(x - M) == mask * exp(x) for
M = 0 / 1000, since exp(-1000+x) underflows to exactly 0 in f32):
    S1_b = sum_l exp(c_bl - 1000*yf_bl)
    S2_b = sum_k exp(-c_bk - 1000*(1-yf_bk))
so the device work per batch shard is ONE exp over 2*L values and a
per-partition accumulation, which the ACT engine does in a single
fused instruction.

Sharding: B=32 rows split 4-per-core across 8 cores. Host-side packing
per core builds a [128, 128] f32 tile: partitions 16*b+q (q in 0..15)
hold the S1 exponents of row b (128 values each), partitions
64+16*b+q hold the S2 exponents. One DMA in, one
activation(Exp, accum_out) producing a [128, 1] column of partial
sums, one DMA out. The host reduces the 16 partials per (row, side),
applies 1/(n_pos*n_neg) and the batch mean.

Written in raw Bass (explicit semaphores): the TileContext tail drain
emits a multi-wait Drain instruction that this container's walrus
rejects ("Too many sync wait commands").

Latency-oriented choices (the kernel is ~6 us of fixed DMA/sem latency):
  - everything rides ONE 64 KB input DMA (512 B per partition,
    full-rate descriptors); a second DMA on any queue serializes
    ~0.8-1.5 us through the DGE/DMA pipe stages.
  - the Bass() constructor preamble (4 const memsets + all-engine
    barrier, ~900 ns) is stripped; the only constant needed (a zero
    bias column for the Exp activation) is memset by the otherwise
    idle Pool engine, sem-guarded off the critical path.
  - a throwaway exp on garbage runs on ACT before the input arrives so
    the hardware Exp table load happens under the DMA wait.
"""

import sys
from contextlib import ExitStack

import numpy as np

for _p in ("/opt/trn_rl_repo",):
    if _p not in sys.path:
        sys.path.append(_p)

B, L = 32, 2048
N_CORES = 8
B_SHARD = B // N_CORES  # 4 batch rows per core
QH = 16  # partitions per (row, side): 4 rows * 2 sides * 16 = 128
JW = L // QH  # 128 free elements per partition
MASK_M = 1000.0  # exp(x - 1000) == 0.0 in f32 for any |x| <= ~870

_CACHE = {}


def _strip_preamble(nc):
    """Remove the const-AP memsets and the all-engine barrier that
    bass.Bass() emits at construction (~900 ns on the critical path).
    Nothing in this kernel reads the const APs, and all cross-engine
    ordering is provided by this kernel's own semaphores."""
    bb0 = nc.m.functions[0].blocks[0]
    insts = bb0.instructions
    keep = [i for i in insts if type(i).__name__ in ("InstCall", "InstRegisterMove")]
    while insts:
        insts.pop()
    for i in keep:
        insts.append(i)


def _build_bass():
    import concourse.bass as bass
    from concourse import mybir

    F32 = mybir.dt.float32
    Exp = mybir.ActivationFunctionType.Exp

    nc = bass.Bass()
    _strip_preamble(nc)

    cm_in = nc.declare_dram_parameter("cm", [128, JW], F32, isOutput=False)
    out = nc.declare_dram_parameter("acc", [128, 1], F32, isOutput=True)

    with ExitStack() as es:
        cm_sb = es.enter_context(nc.sbuf_tensor([128, JW], F32))
        e_junk = es.enter_context(nc.sbuf_tensor([128, JW], F32))
        acc = es.enter_context(nc.sbuf_tensor([128, 1], F32))
        bias0 = es.enter_context(nc.sbuf_tensor([128, 1], F32))

        cm_sem = es.enter_context(nc.semaphore("cm_sem"))
        bias_sem = es.enter_context(nc.semaphore("bias_sem"))
        act_sem = es.enter_context(nc.semaphore("act_sem"))
        out_sem = es.enter_context(nc.semaphore("out_sem"))

        block = es.enter_context(nc.Block())

        @block.sync
        def _(sync):
            sync.dma_start(out=cm_sb[:], in_=cm_in[:]).then_inc(cm_sem, 16)
            sync.wait_ge(act_sem, 1)
            sync.dma_start(out=out[:], in_=acc[:]).then_inc(out_sem, 16)
            sync.wait_ge(out_sem, 16)

        @block.scalar
        def _(scalar):
            # Throwaway exp: forces the hardware Exp table load while the
            # input DMA is still in flight. Reads/writes garbage, never read.
            scalar.activation(out=e_junk[:], in_=e_junk[:], func=Exp, bias=bias0[:])
            scalar.wait_ge(bias_sem, 1)
            scalar.wait_ge(cm_sem, 16)
            # acc[p] = sum_j exp(cm[p, j])
            scalar.activation(
                out=e_junk[:], in_=cm_sb[:], func=Exp, bias=bias0[:],
                accum_out=acc[:],
            ).then_inc(act_sem, 1)

        @block.gpsimd
        def _(gpsimd):
            gpsimd.memset(bias0[:], 0.0)
            gpsimd.drain().then_inc(bias_sem, 1)

    return nc


def _get_nc():
    if "nc" not in _CACHE:
        _CACHE["nc"] = _build_bass()
    return _CACHE["nc"]


def _pack(c, y):
    """Per-core host packing: [4,2048] c + 0/1 y -> [128, 128] f32 of
    mask-folded exponents (see module docstring)."""
    c = np.asarray(c, dtype=np.float32)
    yf = (np.asarray(y) == 1).astype(np.float32)
    cm1 = c - MASK_M * yf  # exp() -> ybar * e^c
    cm2 = -c - MASK_M * (1.0 - yf)  # exp() -> yf * e^-c
    top = cm1.reshape(B_SHARD * QH, JW)
    bot = cm2.reshape(B_SHARD * QH, JW)
    return np.ascontiguousarray(np.concatenate([top, bot], axis=0))


def _run_device(c, y, trace=False):
    from concourse.bass_utils import run_bass_kernel_spmd

    c = np.asarray(c)
    y = np.asarray(y)
    in_maps = [
        {"cm": _pack(c[i * B_SHARD : (i + 1) * B_SHARD],
                     y[i * B_SHARD : (i + 1) * B_SHARD])}
        for i in range(N_CORES)
    ]
    return run_bass_kernel_spmd(
        _get_nc(), in_maps, core_ids=list(range(N_CORES)), trace=trace
    )


def _combine(results, y):
    """results: per-core dicts with 'acc' [128, 1] f32. y: full [32, 2048]."""
    n_pos = (np.asarray(y) == 1).sum(axis=1).astype(np.float64)  # [B]
    n_neg = L - n_pos
    total = 0.0
    for i, r in enumerate(results):
        acc = r["acc"].astype(np.float64).reshape(2, B_SHARD, QH).sum(axis=2)
        s1, s2 = acc[0], acc[1]  # [B_SHARD] each
        b = slice(i * B_SHARD, (i + 1) * B_SHARD)
        total += float(np.sum(s1 * s2 / (n_pos[b] * n_neg[b])))
    return np.float32(total / B)


def kernel(c, y):
    y = np.asarray(y)
    res = _run_device(np.asarray(c), y)
    return np.asarray(_combine(res.results, y), dtype=np.float32)


# revision 12
# speedup vs baseline: 1.4217x; 1.1000x over previous
"""BP-MLL loss kernel for Trainium2, data-parallel over 8 NeuronCores.

Math: the reference loss is
    L = mean_b  (1/(n_pos_b * n_neg_b)) * sum_{k in Y_b, l in Ybar_b} exp(c_bl - c_bk)
The pairwise sum is separable:
    sum_{k,l} yf_k * ybar_l * exp(c_l) * exp(-c_k)
      = (sum_l ybar_l * exp(c_l)) * (sum_k yf_k * exp(-c_k))  =  S1_b * S2_b
so each batch row only needs two masked exp-sums (O(L) instead of O(L^2)).

The masks fold into the exponent (exp(x - M) == mask * exp(x) for
M = 0 / 1000, since exp(-1000+x) underflows to exactly 0 in f32):
    S1_b = sum_l exp(c_bl - 1000*yf_bl)
    S2_b = sum_k exp(-c_bk - 1000*(1-yf_bk))
so the device work per batch shard is ONE exp over 2*L values and a
per-partition accumulation, which the ACT engine does in a single
fused instruction.

Sharding: B=32 rows split 4-per-core across 8 cores. Host-side packing
per core builds a [128, 128] f32 tile: partitions 16*b+q (q in 0..15)
hold the S1 exponents of row b (128 values each), partitions
64+16*b+q hold the S2 exponents. One DMA in, one
activation(Exp, accum_out) producing a [128, 1] column of partial
sums, one DMA out. The host reduces the 16 partials per (row, side),
applies 1/(n_pos*n_neg) and the batch mean.

Written in raw Bass (explicit semaphores): the TileContext tail drain
emits a multi-wait Drain instruction that this container's walrus
rejects ("Too many sync wait commands").

Latency-oriented choices (the kernel is ~6 us of fixed DMA/sem latency):
  - everything rides ONE 64 KB input DMA (512 B per partition,
    full-rate descriptors); a second DMA on any queue serializes
    ~0.8-1.5 us through the DGE/DMA pipe stages.
  - the Bass() constructor preamble (4 const memsets + all-engine
    barrier, ~900 ns) is stripped; the only constant needed (a zero
    bias column for the Exp activation) is memset by the otherwise
    idle Pool engine, sem-guarded off the critical path.
  - a throwaway exp on garbage runs on ACT before the input arrives so
    the hardware Exp table load happens under the DMA wait.
"""

import sys
from contextlib import ExitStack

import numpy as np

for _p in ("/opt/trn_rl_repo",):
    if _p not in sys.path:
        sys.path.append(_p)

B, L = 32, 2048
N_CORES = 8
B_SHARD = B // N_CORES  # 4 batch rows per core
QH = 16  # partitions per (row, side): 4 rows * 2 sides * 16 = 128
JW = L // QH  # 128 free elements per partition
MASK_M = 1000.0  # exp(x - 1000) == 0.0 in f32 for any |x| <= ~870

_CACHE = {}


def _strip_preamble(nc):
    """Remove the const-AP memsets and the all-engine barrier that
    bass.Bass() emits at construction (~900 ns on the critical path).
    Nothing in this kernel reads the const APs, and all cross-engine
    ordering is provided by this kernel's own semaphores."""
    bb0 = nc.m.functions[0].blocks[0]
    insts = bb0.instructions
    keep = [i for i in insts if type(i).__name__ in ("InstCall", "InstRegisterMove")]
    while insts:
        insts.pop()
    for i in keep:
        insts.append(i)


def _strip_regmoves(nc):
    """Drop the per-engine register-preset moves (imm 0 / 0xffffffff)
    from the entry block; nothing in this kernel's instruction stream
    reads those registers."""
    bb0 = nc.m.functions[0].blocks[0]
    insts = bb0.instructions
    keep = [i for i in insts if type(i).__name__ == "InstCall"]
    while insts:
        insts.pop()
    for i in keep:
        insts.append(i)


def _strip_end_barrier(nc):
    """Drop the Block-exit all-engine barrier (drain + event-semaphore
    handshake). Each engine halts on its own; the output DMA is already
    guaranteed complete by the explicit out_sem wait on SP."""
    for bb in nc.m.functions[0].blocks:
        if bb.name.endswith("_end"):
            insts = bb.instructions
            while insts:
                insts.pop()


def _build_bass():
    import concourse.bass as bass
    from concourse import mybir

    F32 = mybir.dt.float32
    Exp = mybir.ActivationFunctionType.Exp

    nc = bass.Bass()
    _strip_preamble(nc)

    cm_in = nc.declare_dram_parameter("cm", [128, JW], F32, isOutput=False)
    out = nc.declare_dram_parameter("acc", [128, 1], F32, isOutput=True)

    with ExitStack() as es:
        cm_sb = es.enter_context(nc.sbuf_tensor([128, JW], F32))
        e_junk = es.enter_context(nc.sbuf_tensor([128, JW], F32))
        acc = es.enter_context(nc.sbuf_tensor([128, 1], F32))
        bias0 = es.enter_context(nc.sbuf_tensor([128, 1], F32))

        cm_sem = es.enter_context(nc.semaphore("cm_sem"))
        bias_sem = es.enter_context(nc.semaphore("bias_sem"))
        act_sem = es.enter_context(nc.semaphore("act_sem"))
        out_sem = es.enter_context(nc.semaphore("out_sem"))

        block = es.enter_context(nc.Block())

        @block.sync
        def _(sync):
            sync.dma_start(out=cm_sb[:], in_=cm_in[:]).then_inc(cm_sem, 16)
            sync.wait_ge(act_sem, 1)
            sync.dma_start(out=out[:], in_=acc[:]).then_inc(out_sem, 16)
            sync.wait_ge(out_sem, 16)

        @block.scalar
        def _(scalar):
            # Throwaway exp: forces the hardware Exp table load while the
            # input DMA is still in flight. Reads/writes garbage, never read.
            scalar.activation(out=e_junk[:], in_=e_junk[:], func=Exp, bias=bias0[:])
            scalar.wait_ge(bias_sem, 1)
            scalar.wait_ge(cm_sem, 16)
            # acc[p] = sum_j exp(cm[p, j])
            scalar.activation(
                out=e_junk[:], in_=cm_sb[:], func=Exp, bias=bias0[:],
                accum_out=acc[:],
            ).then_inc(act_sem, 1)

        @block.gpsimd
        def _(gpsimd):
            gpsimd.memset(bias0[:], 0.0)
            gpsimd.drain().then_inc(bias_sem, 1)

    _strip_regmoves(nc)
    _strip_end_barrier(nc)
    return nc


def _get_nc():
    if "nc" not in _CACHE:
        _CACHE["nc"] = _build_bass()
    return _CACHE["nc"]


def _pack(c, y):
    """Per-core host packing: [4,2048] c + 0/1 y -> [128, 128] f32 of
    mask-folded exponents (see module docstring)."""
    c = np.asarray(c, dtype=np.float32)
    yf = (np.asarray(y) == 1).astype(np.float32)
    cm1 = c - MASK_M * yf  # exp() -> ybar * e^c
    cm2 = -c - MASK_M * (1.0 - yf)  # exp() -> yf * e^-c
    top = cm1.reshape(B_SHARD * QH, JW)
    bot = cm2.reshape(B_SHARD * QH, JW)
    return np.ascontiguousarray(np.concatenate([top, bot], axis=0))


def _run_device(c, y, trace=False):
    from concourse.bass_utils import run_bass_kernel_spmd

    c = np.asarray(c)
    y = np.asarray(y)
    in_maps = [
        {"cm": _pack(c[i * B_SHARD : (i + 1) * B_SHARD],
                     y[i * B_SHARD : (i + 1) * B_SHARD])}
        for i in range(N_CORES)
    ]
    return run_bass_kernel_spmd(
        _get_nc(), in_maps, core_ids=list(range(N_CORES)), trace=trace
    )


def _combine(results, y):
    """results: per-core dicts with 'acc' [128, 1] f32. y: full [32, 2048]."""
    n_pos = (np.asarray(y) == 1).sum(axis=1).astype(np.float64)  # [B]
    n_neg = L - n_pos
    total = 0.0
    for i, r in enumerate(results):
        acc = r["acc"].astype(np.float64).reshape(2, B_SHARD, QH).sum(axis=2)
        s1, s2 = acc[0], acc[1]  # [B_SHARD] each
        b = slice(i * B_SHARD, (i + 1) * B_SHARD)
        total += float(np.sum(s1 * s2 / (n_pos[b] * n_neg[b])))
    return np.float32(total / B)


def kernel(c, y):
    y = np.asarray(y)
    res = _run_device(np.asarray(c), y)
    return np.asarray(_combine(res.results, y), dtype=np.float32)
